# revision 58
# baseline (speedup 1.0000x reference)
"""BWGNN (Bernstein-polynomial graph conv, D=2) on 8 Trainium2 NeuronCores.

Key algebra: inside each polyconv the feat sequence f0, f1=f0-A f0, f2=f1-A f1
is theta-independent, so the device needs only TWO SpMMs (not six), and the
Bernstein mixing folds into W3 on the host:
    out = relu([f0|f1|f2] @ W3' + b3) @ W4 + b4,
    W3'[k*H+i, j] = sum_t theta[t][k] * W3[t*H+i, j].

Distribution (8 cores, SPMD single program):
- nodes row-sharded: core c owns rows [c*12500, (c+1)*12500), padded to 12544.
- node-feature tables are ROW-major fp16 [128, NB*H]; flat row (p, b) holds
  node rl = b*128+p.  The table ships in TWO AllGather halves split by
  source-block range ({0..47}, {48..97}) so each half can fire as soon as the
  producing compute finishes and the next consumer only ever waits on the
  half it reads -- the gather stream never drains at phase boundaries.
- SpMM gathers PAIRS of table rows (256B = two nodes' 64 fp16 feats) per
  edge; a [128, 2] val-mask (val at the edge's parity half, 0 at the other)
  folds value scaling AND pair selection into one DVE multiply.
- matmuls are ROW-major: out[dest, feat] = S^T @ G with lhsT=S (one-hot
  dest-row selector, 128 wide, built per (block, run) with a parity-offset
  iota so chunks spanning two blocks just get two full-partition matmuls).
- each pass runs as two sub-passes (source half 0 for all groups, then half
  1) accumulating half-0 results into an SBUF partial, so sub-pass 1's
  gathers are the only ops waiting on the second AllGather half.
- edges are packed per (4-block group, half) run: per-(block,half) slot
  budgets are equalized across cores (SPMD); chunks are not rounded per
  block (~10% padding instead of ~25%).
"""
import math
import numpy as np

import concourse.bass as bass
import concourse.bacc as bacc
import concourse.mybir as mybir
from concourse.tile import TileContext
from concourse.masks import make_identity
from concourse import bass_utils

N = 100000
F_IN = 128
H = 64
NCLS = 2
D = 2
W = 8                   # cores
R = 12500               # real rows per core
RP = 12544              # padded rows per core (98 * 128)
NB = 98                 # dest blocks per core
NQ = 2                  # source halves (block ranges)
HB0 = 48                # blocks in half 0 (even => pairs don't straddle)
HBS = [0, HB0, NB]      # half boundaries
QS = [W * 128 * HB0 // 2, W * 128 * (NB - HB0) // 2]   # pairs per half
GSC = 4                 # dest blocks per group
PCH = 256               # dense-layer column chunk (2 blocks)
F16 = mybir.dt.float16
F32 = mybir.dt.float32
I16 = mybir.dt.int16


def _theta2():
    P = np.polynomial.polynomial
    thetas = []
    for i in range(D + 1):
        beta = math.factorial(i) * math.factorial(D - i) / math.factorial(D + 1)
        c = P.polymul(P.polypow([0.0, 0.5], i), P.polypow([1.0, -0.5], D - i)) / beta
        c = np.pad(c, (0, D + 1 - len(c)))
        thetas.append(c.astype(np.float64))
    return thetas


def _prep_schedule(counts):
    """counts: [W, NB, NQ] per-core edge counts -> static schedule.

    groups entries (bs, sbase, tcount, runs, off):
      runs = [(q, qcb0, nch, goff)], off[(b, q)] = slot offset within run.
    """
    L = counts.max(axis=0).astype(np.int64)
    for b in range(NB):
        if L[b].sum() == 0:
            L[b, 0] = 1
    groups = []
    sbase = 0
    qcb = np.zeros(NQ, dtype=np.int64)
    for g0 in range(0, NB, GSC):
        bs = list(range(g0, min(g0 + GSC, NB)))
        runs = []
        off = {}
        goff = 0
        for q in range(NQ):
            tot = 0
            for b in bs:
                off[(b, q)] = tot
                tot += int(L[b, q])
            nch = (tot + 127) // 128
            if nch == 0:
                continue
            runs.append((q, int(qcb[q]), nch, goff))
            qcb[q] += nch
            goff += nch
        groups.append((bs, sbase, goff, runs, off))
        sbase += goff
    return L, groups, sbase, [int(qcb[q]) for q in range(NQ)]


def _balance_nodes(adj_rows, adj_cols):
    """Within-core node permutation flattening per-(block, half) in-degree
    sums across cores.  Nodes stay within their source half, so edge half
    assignments are invariant and no iteration is needed.  Returns
    newpos[W, R] (new padded-position of local node rl)."""
    core = adj_rows // R
    rloc = adj_rows - core * R
    qe = ((adj_cols % R) // 128 >= HB0).astype(np.int64)
    d = np.zeros((W, R, NQ), dtype=np.int64)
    np.add.at(d, (core, rloc, qe), 1)
    newpos = np.zeros((W, R), dtype=np.int64)
    for c in range(W):
        for b0, b1 in [(0, HB0), (HB0, NB)]:
            lo, hi = b0 * 128, min(b1 * 128, R)
            dd = d[c, lo:hi]
            order = np.lexsort((dd[:, 0] - dd[:, 1],
                                -(dd[:, 0] + dd[:, 1])))
            caps = [min((b + 1) * 128, R) - b * 128 for b in range(b0, b1)]
            seq = []
            fill = [0] * len(caps)
            rev = False
            while len(seq) < hi - lo:
                rng = range(len(caps) - 1, -1, -1) if rev else range(len(caps))
                for bi in rng:
                    if fill[bi] < caps[bi]:
                        seq.append((bi, fill[bi]))
                        fill[bi] += 1
                rev = not rev
            bi_arr = np.array([s[0] for s in seq], dtype=np.int64)
            sl_arr = np.array([s[1] for s in seq], dtype=np.int64)
            newpos[c, lo + order] = (b0 + bi_arr) * 128 + sl_arr
    return newpos


def _prep_edges(adj_rows, adj_cols, adj_vals):
    """Per-core gather indices / dest-row vectors / val-masks in slot order.

    Source node (c, rl) at padded position np=newpos[c,rl], p=np%128,
    b=np//128 lives at flat row (c*128+p)*HBn + (b-b0) of its half's
    table; pair = row//2, parity = b%2 (half sizes even)."""
    newpos = _balance_nodes(adj_rows, adj_cols)
    core = adj_rows // R
    rloc = adj_rows - core * R
    dpos = newpos[core, rloc]
    blk = dpos // 128
    rowin = dpos % 128
    csrc = adj_cols // R
    rsrc = adj_cols - csrc * R
    spos = newpos[csrc, rsrc]
    bsrc = spos // 128
    cp = csrc * 128 + spos % 128
    q = (bsrc >= HB0).astype(np.int64)
    hbn = np.where(q == 0, HB0, NB - HB0)
    row = cp * hbn + bsrc - np.where(q == 0, 0, HB0)
    pair = row // 2
    parity = bsrc % 2
    qoff = pair

    counts = np.zeros((W, NB, NQ), dtype=np.int64)
    np.add.at(counts, (core, blk, q), 1)
    L, groups, T, Tq = _prep_schedule(counts)

    off_arr = np.zeros((NB, NQ), dtype=np.int64)
    runbase = np.full((NB, NQ), -1, dtype=np.int64)
    qchunkbase = np.zeros((NB, NQ), dtype=np.int64)
    for bs, sbase, tcount, runs, off in groups:
        for (qq, qcb0, nch, goff) in runs:
            for b in bs:
                off_arr[b, qq] = off[(b, qq)]
                runbase[b, qq] = sbase + goff
                qchunkbase[b, qq] = qcb0

    order = np.lexsort((qoff, blk, q, core))
    sc, sb_, sq = core[order], blk[order], q[order]
    s_qoff, s_rowin = qoff[order], rowin[order]
    s_par, s_val = parity[order], adj_vals[order]

    m = len(sc)
    key = (sc * NB + sb_) * NQ + sq
    brk = np.nonzero(np.diff(key))[0] + 1
    starts = np.concatenate([[0], brk])
    lens = np.diff(np.concatenate([starts, [m]]))
    pos = np.arange(m) - np.repeat(starts, lens)

    runslot = off_arr[sb_, sq] + pos
    jq = runslot // 128
    p = runslot % 128
    tglob = runbase[sb_, sq] + jq
    qch = qchunkbase[sb_, sq] + jq
    assert runbase[sb_, sq].min() >= 0
    assert int(qoff.max()) < max(QS) <= 32768

    rowv = np.zeros((W, 128, T), dtype=np.float16)
    vm = np.zeros((W, 128, 2 * T), dtype=np.float16)
    idx16 = [[np.zeros(Tq[qq] * 128, dtype=np.int16) for qq in range(NQ)]
             for _ in range(W)]
    s_rowd = s_rowin + 128 * (sb_ % 2)      # parity-disambiguated dest row
    for c in range(W):
        sel = sc == c
        rowv[c, p[sel], tglob[sel]] = s_rowd[sel].astype(np.float16)
        vm[c, p[sel], 2 * tglob[sel] + s_par[sel]] = s_val[sel].astype(np.float16)
        for qq in range(NQ):
            s2 = sel & (sq == qq)
            idx16[c][qq][qch[s2] * 128 + p[s2]] = s_qoff[s2].astype(np.int16)

    idx_wrapped = []
    for c in range(W):
        parts = []
        for qq in range(NQ):
            a = idx16[c][qq]
            parts.append(a.reshape(len(a) // 16, 16).T)
        cat = np.concatenate(parts, axis=1)
        idx_wrapped.append(np.tile(cat, (8, 1)).copy())
    return idx_wrapped, rowv, vm, groups, L, T, Tq, newpos


def _build(groups, L, T, Tq):
    qbase = [8 * sum(Tq[:qq]) for qq in range(NQ)]
    gmax = max(nch for _, _, _, runs, _ in groups for (_, _, nch, _) in runs)
    QCH = 3                              # chunks per gather call (queue spread)
    SMX = 1 + max((int(L[b, q]) + 127) // 128
                  for b in range(NB) for q in range(NQ))
    CW = [HB0 * H, (NB - HB0) * H]       # rm column widths of the two halves

    nc = bacc.Bacc("TRN2", num_swdge_queues=4)
    rg = [list(range(W))]

    xT = nc.dram_tensor("xT", [F_IN, RP], F16, kind="ExternalInput")
    w1 = nc.dram_tensor("w1", [F_IN, H], F16, kind="ExternalInput")
    w2 = nc.dram_tensor("w2", [H, H], F16, kind="ExternalInput")
    w3 = nc.dram_tensor("w3", [3 * H, H], F16, kind="ExternalInput")
    w4 = nc.dram_tensor("w4", [H, NCLS], F16, kind="ExternalInput")
    b1 = nc.dram_tensor("b1", [H, 1], F32, kind="ExternalInput")
    b2 = nc.dram_tensor("b2", [H, 1], F32, kind="ExternalInput")
    b3 = nc.dram_tensor("b3", [H, 1], F32, kind="ExternalInput")
    b4 = nc.dram_tensor("b4", [NCLS, 1], F32, kind="ExternalInput")
    idx_t = nc.dram_tensor("idx", [128, T * 8], I16, kind="ExternalInput")
    rowv_t = nc.dram_tensor("rowv", [128, T], F16, kind="ExternalInput")
    vm_t = nc.dram_tensor("vm", [128, 2 * T], F16, kind="ExternalInput")
    iota_t = nc.dram_tensor("iota", [128, 256], F16, kind="ExternalInput")
    out_t = nc.dram_tensor("out", [NCLS, RP], F32, kind="ExternalOutput")

    ag_in = [[nc.dram_tensor(f"agin{i}{h}", [128, CW[h]], F16, kind="Internal")
              for h in range(2)] for i in range(2)]
    ag_out = [[nc.dram_tensor(f"agout{i}{h}", [W * 128, CW[h]], F16,
                              kind="Internal", addr_space="Shared")
               for h in range(2)] for i in range(2)]

    with TileContext(nc) as tc:
        with tc.tile_pool(name="c0", bufs=1) as cpool, \
             tc.tile_pool(name="mm", bufs=3) as mpool, \
             tc.tile_pool(name="gg", bufs=6) as gpool, \
             tc.tile_pool(name="sl", bufs=8) as slpool, \
             tc.tile_pool(name="sb", bufs=12) as subpool, \
             tc.tile_pool(name="ps", bufs=2, space="PSUM") as pspool, \
             tc.tile_pool(name="pb", bufs=2, space="PSUM") as pbpool, \
             tc.tile_pool(name="pk", bufs=4, space="PSUM") as pkpool:

            ident = cpool.tile([128, 128], F16)
            make_identity(nc, ident[:])

            # PE pre-warm: ~5us of junk matmuls lifts the HAM clock gate to
            # 2.4 GHz before the MLP stream begins (see tensor-engine docs).
            warm = pkpool.tile([128, 128], F32, tag="pblk", name="pblk0",
                               space="PSUM")
            for _ in range(24):
                nc.tensor.matmul(warm[:], lhsT=ident[:], rhs=ident[:],
                                 start=True, stop=True)

            def load_const(name, src, shape, dt, eng=None):
                tile = cpool.tile(shape, dt, tag=name)
                (eng or nc.sync).dma_start(out=tile[:], in_=src)
                return tile

            w1_sb = load_const("w1", w1[:], [F_IN, H], F16, eng=nc.scalar)
            w2_sb = load_const("w2", w2[:], [H, H], F16, eng=nc.scalar)
            w3ab_sb = load_const("w3ab", w3[0:128, :], [128, H], F16,
                                 eng=nc.scalar)
            w3c_sb = load_const("w3c", w3[128:192, :], [H, H], F16,
                                 eng=nc.scalar)
            w4_sb = load_const("w4", w4[:], [H, NCLS], F16, eng=nc.scalar)
            b1_sb = load_const("b1", b1[:], [H, 1], F32, eng=nc.scalar)
            b2_sb = load_const("b2", b2[:], [H, 1], F32, eng=nc.scalar)
            b3_sb = load_const("b3", b3[:], [H, 1], F32, eng=nc.scalar)
            b4_sb = load_const("b4", b4[:], [NCLS, 1], F32, eng=nc.scalar)
            # big SpMM consts go via the Activation HWDGE queue so phase-A
            # xT chunk loads on the sync queue aren't stuck behind them
            idx_sb = load_const("idx", idx_t[:], [128, T * 8], I16,
                                eng=nc.scalar)
            rowv_sb = load_const("rowv", rowv_t[:], [128, T], F16,
                                 eng=nc.scalar)
            vm_sb = load_const("vm", vm_t[:], [128, 2 * T], F16,
                               eng=nc.scalar)
            iota_sb = load_const("iota", iota_t[:], [128, 256], F16,
                                 eng=nc.scalar)

            h1_f2 = cpool.tile([128, RP], F16)   # h1, then f2 (fm) on p0..63
            h_cat = cpool.tile([128, RP], F16)   # f0 fm p0..63, f1 fm p64..127
            f_rm0 = cpool.tile([128, NB * H], F16)
            f_rm1 = cpool.tile([128, NB * H], F16)
            part = cpool.tile([128, NB * H], F16)  # sub-pass-0 partial A.f

            def ship(src_rm, i, h):
                c0, c1 = HBS[h] * H, HBS[h + 1] * H
                nc.sync.dma_start(out=ag_in[i][h][:], in_=src_rm[:, c0:c1])
                nc.gpsimd.collective_compute(
                    "AllGather", mybir.AluOpType.bypass, replica_groups=rg,
                    ins=[ag_in[i][h][:]], outs=[ag_out[i][h][:]])

            # ---------- MLP1 + MLP2 + ship f0 (row-major fp16) ----------
            for o in range(0, RP, PCH):
                pc = min(PCH, RP - o)
                xt = mpool.tile([F_IN, PCH], F16, tag="xin")
                nc.sync.dma_start(out=xt[:, :pc], in_=xT[:, o:o + pc])
                pt = pspool.tile([H, PCH], F32, tag="pmlp", space="PSUM")
                nc.tensor.matmul(pt[:, :pc], lhsT=w1_sb[:], rhs=xt[:, :pc],
                                 start=True, stop=True)
                nc.vector.tensor_scalar(
                    out=h1_f2[0:H, o:o + pc], in0=pt[:, :pc],
                    scalar1=b1_sb[:], scalar2=0.0,
                    op0=mybir.AluOpType.add, op1=mybir.AluOpType.max)
                pt2 = pspool.tile([H, PCH], F32, tag="pmlp", space="PSUM")
                nc.tensor.matmul(pt2[:, :pc], lhsT=w2_sb[:],
                                 rhs=h1_f2[0:H, o:o + pc],
                                 start=True, stop=True)
                nc.scalar.activation(h_cat[0:H, o:o + pc], pt2[:, :pc],
                                     mybir.ActivationFunctionType.Relu,
                                     bias=b2_sb[:], scale=1.0)
                for b in range(o // 128, (o + pc) // 128):
                    ptr = pbpool.tile([128, 128], F16, tag="ptr", space="PSUM")
                    nc.tensor.transpose(ptr[0:128, 0:H],
                                        h_cat[0:H, b * 128:(b + 1) * 128],
                                        ident[0:H, 0:H])
                    nc.vector.tensor_copy(f_rm0[:, b * H:(b + 1) * H],
                                          ptr[0:128, 0:H])
                if (o + pc) // 128 == HB0:
                    ship(f_rm0, 0, 0)
            ship(f_rm0, 0, 1)

            # ---------- SpMM pass (two sub-passes over source halves) ----
            qrr = [0]

            def spmm(srcs, cur_rm, nxt_rm, fm_out, fm_p, ship_i=None,
                     tail_cb=None):
                qv = []
                for h in range(2):
                    flat = srcs[h][:].rearrange("p x -> (p x)")
                    qv.append(flat.rearrange("(r s) -> r s", s=128))

                def do_run(bs, sbase, runs, off, qq):
                    run = [r for r in runs if r[0] == qq]
                    if not run:
                        return None, None
                    (_, qcb0, nch, goff) = run[0]
                    gt = gpool.tile([128, gmax * 128], F16, tag="gbuf")
                    g16 = gpool.tile([128, gmax * 128], F16, tag="g16")
                    for s0 in range(0, nch, QCH):
                        ns = min(QCH, nch - s0)
                        nc.gpsimd.dma_gather(
                            out_ap=gt[:, s0 * 128:(s0 + ns) * 128]
                            .rearrange("p (t e) -> p t e", e=128),
                            in_ap=qv[qq],
                            idxs_ap=idx_sb[:, qbase[qq] + 8 * (qcb0 + s0):
                                           qbase[qq] + 8 * (qcb0 + s0 + ns)],
                            num_idxs=ns * 128,
                            num_idxs_reg=ns * 128,
                            elem_size=128,
                            single_packet=True,
                            queue_num=qrr[0],
                        )
                        qrr[0] = (qrr[0] + 1) % 4
                    nc.vector.tensor_tensor(
                        out=g16[:, :nch * 128]
                        .rearrange("p (t e) -> p t e", e=H),
                        in0=gt[:, :nch * 128]
                        .rearrange("p (t e) -> p t e", e=H),
                        in1=vm_sb[:, 2 * (sbase + goff):
                                  2 * (sbase + goff + nch)]
                        .unsqueeze(2).to_broadcast([128, 2 * nch, H]),
                        op=mybir.AluOpType.mult)
                    return g16, (sbase + goff)

                def blocks_mm(bs, off, qq, g16, rvbase, pts):
                    for a, b in enumerate(bs):
                        if L[b, qq] == 0:
                            continue
                        o0 = off[(b, qq)]
                        jb0 = o0 // 128
                        jb1 = (o0 + int(L[b, qq]) - 1) // 128
                        nj = jb1 - jb0 + 1
                        bp = (b % 2) * 128
                        sblk = slpool.tile([128, SMX * 128], F16, tag="sslab")
                        nc.vector.tensor_tensor(
                            out=sblk[:, :nj * 128]
                            .rearrange("p (t r) -> p t r", r=128),
                            in0=iota_sb[:, bp:bp + 128].unsqueeze(1)
                            .to_broadcast([128, nj, 128]),
                            in1=rowv_sb[:, rvbase + jb0:rvbase + jb0 + nj]
                            .unsqueeze(2).to_broadcast([128, nj, 128]),
                            op=mybir.AluOpType.is_equal)
                        for j in range(jb0, jb1 + 1):
                            nc.tensor.matmul(
                                pts[a][:],
                                lhsT=sblk[:, (j - jb0) * 128:
                                          (j - jb0 + 1) * 128],
                                rhs=g16[:, j * 128:(j + 1) * 128],
                                start=(j == jb0), stop=(j == jb1))

                # sub-pass 0: accumulate half-0 contributions into `part`
                for bs, sbase, tcount, runs, off in groups:
                    g16, rvbase = do_run(bs, sbase, runs, off, 0)
                    pts = [pkpool.tile([128, 128], F32, tag="pblk",
                                       name=f"pblk{ai}", space="PSUM")
                           for ai in range(len(bs))]
                    if g16 is not None:
                        blocks_mm(bs, off, 0, g16, rvbase, pts)
                    for a, b in enumerate(bs):
                        if L[b, 0] == 0:
                            nc.vector.memset(part[:, b * H:(b + 1) * H], 0)
                            continue
                        nc.scalar.copy(part[:, b * H:(b + 1) * H],
                                       pts[a][:, 0:H])
                        nc.vector.tensor_tensor(
                            out=part[:, b * H:(b + 1) * H],
                            in0=part[:, b * H:(b + 1) * H],
                            in1=pts[a][:, H:128],
                            op=mybir.AluOpType.add)

                # sub-pass 1: half-1 contributions, final combine + fm copy
                for gi, (bs, sbase, tcount, runs, off) in enumerate(groups):
                    g16, rvbase = do_run(bs, sbase, runs, off, 1)
                    pts = [pkpool.tile([128, 128], F32, tag="pblk",
                                       name=f"pblk{ai}", space="PSUM")
                           for ai in range(len(bs))]
                    if g16 is not None:
                        blocks_mm(bs, off, 1, g16, rvbase, pts)
                    for a, b in enumerate(bs):
                        ts = subpool.tile([128, H], F16, tag="tsub")
                        nc.vector.tensor_tensor(
                            out=ts[:],
                            in0=cur_rm[:, b * H:(b + 1) * H],
                            in1=part[:, b * H:(b + 1) * H],
                            op=mybir.AluOpType.subtract)
                        if L[b, 1] > 0:
                            t2 = subpool.tile([128, H], F16, tag="tsu2")
                            nc.vector.tensor_tensor(
                                out=t2[:], in0=ts[:], in1=pts[a][:, 0:H],
                                op=mybir.AluOpType.subtract)
                            nc.vector.tensor_tensor(
                                out=nxt_rm[:, b * H:(b + 1) * H],
                                in0=t2[:], in1=pts[a][:, H:128],
                                op=mybir.AluOpType.subtract)
                        else:
                            nc.vector.tensor_copy(
                                nxt_rm[:, b * H:(b + 1) * H], ts[:])
                        ptr = pbpool.tile([128, 128], F16, tag="ptr",
                                          space="PSUM")
                        nc.tensor.transpose(
                            ptr[0:H, 0:128],
                            nxt_rm[:, b * H:(b + 1) * H],
                            ident[:])
                        nc.scalar.copy(
                            fm_out[fm_p:fm_p + H, b * 128:(b + 1) * 128],
                            ptr[0:H, 0:128])
                    if ship_i is not None and bs[-1] + 1 == HB0:
                        ship(nxt_rm, ship_i, 0)
                    if tail_cb is not None:
                        tail_cb(bs[-1] + 1)
                if ship_i is not None:
                    ship(nxt_rm, ship_i, 1)

            # ---------- MLP3 + MLP4 (interleaved into pass 2's tail) ----
            PC3 = 256
            m34_done = [0]

            def mlp34_upto(blocks_done):
                hi = blocks_done * 128 // PC3
                for c3 in range(m34_done[0], hi):
                    o = c3 * PC3
                    pt = pspool.tile([H, PC3], F32, tag="pmlp", space="PSUM")
                    nc.tensor.matmul(pt[:], lhsT=w3ab_sb[:],
                                     rhs=h_cat[:, o:o + PC3],
                                     start=True, stop=False)
                    nc.tensor.matmul(pt[:], lhsT=w3c_sb[:],
                                     rhs=h1_f2[0:H, o:o + PC3],
                                     start=False, stop=True)
                    h3 = mpool.tile([H, PC3], F16, tag="h3")
                    nc.scalar.activation(h3[:], pt[:],
                                         mybir.ActivationFunctionType.Relu,
                                         bias=b3_sb[:], scale=1.0)
                    po = pspool.tile([H, PC3], F32, tag="pmlp", space="PSUM")
                    nc.tensor.matmul(po[0:NCLS, :], lhsT=w4_sb[:], rhs=h3[:],
                                     start=True, stop=True)
                    ot = mpool.tile([NCLS, PC3], F32, tag="ot")
                    nc.scalar.activation(ot[:], po[0:NCLS, :],
                                         mybir.ActivationFunctionType.Identity,
                                         bias=b4_sb[:], scale=1.0)
                    nc.sync.dma_start(out=out_t[:, o:o + PC3], in_=ot[:])
                m34_done[0] = hi

            spmm(ag_out[0], f_rm0, f_rm1, h_cat, H, ship_i=1)
            spmm(ag_out[1], f_rm1, f_rm0, h1_f2, 0, tail_cb=mlp34_upto)
            mlp34_upto(NB)

    nc.compile()
    return nc


def _plan(in_feat, adj_rows, adj_cols, adj_vals, W1, b1, W2, b2, W3, b3, W4, b4):
    in_feat = np.asarray(in_feat, dtype=np.float32)
    adj_rows = np.asarray(adj_rows).astype(np.int64)
    adj_cols = np.asarray(adj_cols).astype(np.int64)
    adj_vals = np.asarray(adj_vals, dtype=np.float32)

    thetas = _theta2()
    W3 = np.asarray(W3, dtype=np.float64)
    W3p = np.zeros((3 * H, H), dtype=np.float64)
    for k in range(D + 1):
        for t in range(D + 1):
            W3p[k * H:(k + 1) * H] += thetas[t][k] * W3[t * H:(t + 1) * H]

    idx_wrapped, rowv, vm, groups, L, T, Tq, newpos = _prep_edges(
        adj_rows, adj_cols, adj_vals)

    nc = _build(groups, L, T, Tq)

    iota = np.tile(np.arange(256, dtype=np.float16), (128, 1))
    in_maps = []
    for c in range(W):
        shard = np.zeros((F_IN, RP), dtype=np.float16)
        shard[:, newpos[c]] = in_feat[c * R:(c + 1) * R].T.astype(np.float16)
        in_maps.append({
            "xT": shard,
            "w1": np.asarray(W1).astype(np.float16),
            "w2": np.asarray(W2).astype(np.float16),
            "w3": W3p.astype(np.float16),
            "w4": np.asarray(W4).astype(np.float16),
            "b1": np.asarray(b1, dtype=np.float32).reshape(H, 1),
            "b2": np.asarray(b2, dtype=np.float32).reshape(H, 1),
            "b3": np.asarray(b3, dtype=np.float32).reshape(H, 1),
            "b4": np.asarray(b4, dtype=np.float32).reshape(NCLS, 1),
            "idx": idx_wrapped[c],
            "rowv": rowv[c],
            "vm": vm[c],
            "iota": iota,
        })
    return nc, in_maps, newpos


def kernel(in_feat, adj_rows, adj_cols, adj_vals, W1, b1, W2, b2, W3, b3, W4, b4):
    nc, in_maps, newpos = _plan(in_feat, adj_rows, adj_cols, adj_vals,
                                W1, b1, W2, b2, W3, b3, W4, b4)
    res = bass_utils.run_bass_kernel_spmd(nc, in_maps, list(range(W)))
    out = np.concatenate(
        [res.results[c]["out"][:, newpos[c]].T for c in range(W)], axis=0)
    return np.ascontiguousarray(out, dtype=np.float32)


# revision 60
# speedup vs baseline: 1.1610x; 1.1610x over previous
"""BWGNN (Bernstein-polynomial graph conv, D=2) on 8 Trainium2 NeuronCores.

Key algebra: inside each polyconv the feat sequence f0, f1=f0-A f0, f2=f1-A f1
is theta-independent, so the device needs only TWO SpMMs (not six), and the
Bernstein mixing folds into W3 on the host:
    out = relu([f0|f1|f2] @ W3' + b3) @ W4 + b4,
    W3'[k*H+i, j] = sum_t theta[t][k] * W3[t*H+i, j].

Distribution (8 cores, SPMD single program):
- nodes row-sharded: core c owns rows [c*12500, (c+1)*12500), padded to 12544.
- node-feature tables are ROW-major fp16 [128, NB*H]; flat row (p, b) holds
  node rl = b*128+p.  The table ships in TWO AllGather halves split by
  source-block range ({0..47}, {48..97}) so each half can fire as soon as the
  producing compute finishes and the next consumer only ever waits on the
  half it reads -- the gather stream never drains at phase boundaries.
- SpMM gathers PAIRS of table rows (256B = two nodes' 64 fp16 feats) per
  edge; a [128, 2] val-mask (val at the edge's parity half, 0 at the other)
  folds value scaling AND pair selection into one DVE multiply.
- matmuls are ROW-major: out[dest, feat] = S^T @ G with lhsT=S (one-hot
  dest-row selector, 128 wide, built per (block, run) with a parity-offset
  iota so chunks spanning two blocks just get two full-partition matmuls).
- each pass runs as two sub-passes (source half 0 for all groups, then half
  1) accumulating half-0 results into an SBUF partial, so sub-pass 1's
  gathers are the only ops waiting on the second AllGather half.
- edges are packed per (4-block group, half) run: per-(block,half) slot
  budgets are equalized across cores (SPMD); chunks are not rounded per
  block (~10% padding instead of ~25%).
"""
import math
import numpy as np

import concourse.bass as bass
import concourse.bacc as bacc
import concourse.mybir as mybir
from concourse.tile import TileContext
from concourse.masks import make_identity
from concourse import bass_utils

N = 100000
F_IN = 128
H = 64
NCLS = 2
D = 2
W = 8                   # cores
R = 12500               # real rows per core
RP = 12544              # padded rows per core (98 * 128)
NB = 98                 # dest blocks per core
NQ = 2                  # source halves (block ranges)
HB0 = 48                # blocks in half 0 (even => pairs don't straddle)
HBS = [0, HB0, NB]      # half boundaries
QS = [W * 128 * HB0 // 2, W * 128 * (NB - HB0) // 2]   # pairs per half
GSC = 4                 # dest blocks per group
PCH = 256               # dense-layer column chunk (2 blocks)
F16 = mybir.dt.float16
F32 = mybir.dt.float32
I16 = mybir.dt.int16


def _theta2():
    P = np.polynomial.polynomial
    thetas = []
    for i in range(D + 1):
        beta = math.factorial(i) * math.factorial(D - i) / math.factorial(D + 1)
        c = P.polymul(P.polypow([0.0, 0.5], i), P.polypow([1.0, -0.5], D - i)) / beta
        c = np.pad(c, (0, D + 1 - len(c)))
        thetas.append(c.astype(np.float64))
    return thetas


def _prep_schedule(counts):
    """counts: [W, NB, NQ] per-core edge counts -> static schedule.

    groups entries (bs, sbase, tcount, runs, off):
      runs = [(q, qcb0, nch, goff)], off[(b, q)] = slot offset within run.
    """
    L = counts.max(axis=0).astype(np.int64)
    for b in range(NB):
        if L[b].sum() == 0:
            L[b, 0] = 1
    groups = []
    sbase = 0
    qcb = np.zeros(NQ, dtype=np.int64)
    for g0 in range(0, NB, GSC):
        bs = list(range(g0, min(g0 + GSC, NB)))
        runs = []
        off = {}
        goff = 0
        for q in range(NQ):
            tot = 0
            for b in bs:
                off[(b, q)] = tot
                tot += int(L[b, q])
            nch = (tot + 127) // 128
            if nch == 0:
                continue
            runs.append((q, int(qcb[q]), nch, goff))
            qcb[q] += nch
            goff += nch
        groups.append((bs, sbase, goff, runs, off))
        sbase += goff
    return L, groups, sbase, [int(qcb[q]) for q in range(NQ)]


def _balance_nodes(adj_rows, adj_cols):
    """Within-core node permutation flattening per-(block, half) in-degree
    sums across cores.  Nodes stay within their source half, so edge half
    assignments are invariant and no iteration is needed.  Returns
    newpos[W, R] (new padded-position of local node rl)."""
    core = adj_rows // R
    rloc = adj_rows - core * R
    qe = ((adj_cols % R) // 128 >= HB0).astype(np.int64)
    d = np.zeros((W, R, NQ), dtype=np.int64)
    np.add.at(d, (core, rloc, qe), 1)
    newpos = np.zeros((W, R), dtype=np.int64)
    for c in range(W):
        for b0, b1 in [(0, HB0), (HB0, NB)]:
            lo, hi = b0 * 128, min(b1 * 128, R)
            dd = d[c, lo:hi]
            order = np.lexsort((dd[:, 0] - dd[:, 1],
                                -(dd[:, 0] + dd[:, 1])))
            caps = [min((b + 1) * 128, R) - b * 128 for b in range(b0, b1)]
            seq = []
            fill = [0] * len(caps)
            rev = False
            while len(seq) < hi - lo:
                rng = range(len(caps) - 1, -1, -1) if rev else range(len(caps))
                for bi in rng:
                    if fill[bi] < caps[bi]:
                        seq.append((bi, fill[bi]))
                        fill[bi] += 1
                rev = not rev
            bi_arr = np.array([s[0] for s in seq], dtype=np.int64)
            sl_arr = np.array([s[1] for s in seq], dtype=np.int64)
            newpos[c, lo + order] = (b0 + bi_arr) * 128 + sl_arr
    return newpos


def _prep_edges(adj_rows, adj_cols, adj_vals):
    """Per-core gather indices / dest-row vectors / val-masks in slot order.

    Source node (c, rl) at padded position np=newpos[c,rl], p=np%128,
    b=np//128 lives at flat row (c*128+p)*HBn + (b-b0) of its half's
    table; pair = row//2, parity = b%2 (half sizes even)."""
    newpos = _balance_nodes(adj_rows, adj_cols)
    core = adj_rows // R
    rloc = adj_rows - core * R
    dpos = newpos[core, rloc]
    blk = dpos // 128
    rowin = dpos % 128
    csrc = adj_cols // R
    rsrc = adj_cols - csrc * R
    spos = newpos[csrc, rsrc]
    bsrc = spos // 128
    cp = csrc * 128 + spos % 128
    q = (bsrc >= HB0).astype(np.int64)
    hbn = np.where(q == 0, HB0, NB - HB0)
    row = cp * hbn + bsrc - np.where(q == 0, 0, HB0)
    pair = row // 2
    parity = bsrc % 2
    qoff = pair

    counts = np.zeros((W, NB, NQ), dtype=np.int64)
    np.add.at(counts, (core, blk, q), 1)
    L, groups, T, Tq = _prep_schedule(counts)

    off_arr = np.zeros((NB, NQ), dtype=np.int64)
    runbase = np.full((NB, NQ), -1, dtype=np.int64)
    qchunkbase = np.zeros((NB, NQ), dtype=np.int64)
    for bs, sbase, tcount, runs, off in groups:
        for (qq, qcb0, nch, goff) in runs:
            for b in bs:
                off_arr[b, qq] = off[(b, qq)]
                runbase[b, qq] = sbase + goff
                qchunkbase[b, qq] = qcb0

    order = np.lexsort((qoff, blk, q, core))
    sc, sb_, sq = core[order], blk[order], q[order]
    s_qoff, s_rowin = qoff[order], rowin[order]
    s_par, s_val = parity[order], adj_vals[order]

    m = len(sc)
    key = (sc * NB + sb_) * NQ + sq
    brk = np.nonzero(np.diff(key))[0] + 1
    starts = np.concatenate([[0], brk])
    lens = np.diff(np.concatenate([starts, [m]]))
    pos = np.arange(m) - np.repeat(starts, lens)

    runslot = off_arr[sb_, sq] + pos
    jq = runslot // 128
    p = runslot % 128
    tglob = runbase[sb_, sq] + jq
    qch = qchunkbase[sb_, sq] + jq
    assert runbase[sb_, sq].min() >= 0
    assert int(qoff.max()) < max(QS) <= 32768

    rowv = np.zeros((W, 128, T), dtype=np.float16)
    vm = np.zeros((W, 128, 2 * T), dtype=np.float16)
    idx16 = [[np.zeros(Tq[qq] * 128, dtype=np.int16) for qq in range(NQ)]
             for _ in range(W)]
    s_rowd = s_rowin + 128 * (sb_ % 2)      # parity-disambiguated dest row
    for c in range(W):
        sel = sc == c
        rowv[c, p[sel], tglob[sel]] = s_rowd[sel].astype(np.float16)
        vm[c, p[sel], 2 * tglob[sel] + s_par[sel]] = s_val[sel].astype(np.float16)
        for qq in range(NQ):
            s2 = sel & (sq == qq)
            idx16[c][qq][qch[s2] * 128 + p[s2]] = s_qoff[s2].astype(np.int16)

    idx_wrapped = []
    for c in range(W):
        parts = []
        for qq in range(NQ):
            a = idx16[c][qq]
            parts.append(a.reshape(len(a) // 16, 16).T)
        cat = np.concatenate(parts, axis=1)
        idx_wrapped.append(np.tile(cat, (8, 1)).copy())
    return idx_wrapped, rowv, vm, groups, L, T, Tq, newpos


def _build(groups, L, T, Tq):
    qbase = [8 * sum(Tq[:qq]) for qq in range(NQ)]
    gmax = max(nch for _, _, _, runs, _ in groups for (_, _, nch, _) in runs)
    QCH = 3                              # chunks per gather call (queue spread)
    SMX = 1 + max((int(L[b, q]) + 127) // 128
                  for b in range(NB) for q in range(NQ))
    CW = [HB0 * H, (NB - HB0) * H]       # rm column widths of the two halves

    nc = bacc.Bacc("TRN2", num_swdge_queues=4)
    rg = [list(range(W))]

    xT = nc.dram_tensor("xT", [F_IN, RP], F16, kind="ExternalInput")
    w1 = nc.dram_tensor("w1", [F_IN, H], F16, kind="ExternalInput")
    w2 = nc.dram_tensor("w2", [H, H], F16, kind="ExternalInput")
    w3 = nc.dram_tensor("w3", [3 * H, H], F16, kind="ExternalInput")
    w4 = nc.dram_tensor("w4", [H, NCLS], F16, kind="ExternalInput")
    b1 = nc.dram_tensor("b1", [H, 1], F32, kind="ExternalInput")
    b2 = nc.dram_tensor("b2", [H, 1], F32, kind="ExternalInput")
    b3 = nc.dram_tensor("b3", [H, 1], F32, kind="ExternalInput")
    b4 = nc.dram_tensor("b4", [NCLS, 1], F32, kind="ExternalInput")
    idx_t = nc.dram_tensor("idx", [128, T * 8], I16, kind="ExternalInput")
    rowv_t = nc.dram_tensor("rowv", [128, T], F16, kind="ExternalInput")
    vm_t = nc.dram_tensor("vm", [128, 2 * T], F16, kind="ExternalInput")
    iota_t = nc.dram_tensor("iota", [128, 256], F16, kind="ExternalInput")
    out_t = nc.dram_tensor("out", [NCLS, RP], F32, kind="ExternalOutput")

    ag_in = [[nc.dram_tensor(f"agin{i}{h}", [128, CW[h]], F16, kind="Internal")
              for h in range(2)] for i in range(2)]
    ag_out = [[nc.dram_tensor(f"agout{i}{h}", [W * 128, CW[h]], F16,
                              kind="Internal", addr_space="Shared")
               for h in range(2)] for i in range(2)]

    with TileContext(nc) as tc:
        with tc.tile_pool(name="c0", bufs=1) as cpool, \
             tc.tile_pool(name="mm", bufs=3) as mpool, \
             tc.tile_pool(name="gg", bufs=5) as gpool, \
             tc.tile_pool(name="sl", bufs=6) as slpool, \
             tc.tile_pool(name="sb", bufs=8) as subpool, \
             tc.tile_pool(name="ps", bufs=2, space="PSUM") as pspool, \
             tc.tile_pool(name="pb", bufs=2, space="PSUM") as pbpool, \
             tc.tile_pool(name="pk", bufs=4, space="PSUM") as pkpool:

            ident = cpool.tile([128, 128], F16)
            make_identity(nc, ident[:])

            # PE pre-warm: ~5us of junk matmuls lifts the HAM clock gate to
            # 2.4 GHz before the MLP stream begins (see tensor-engine docs).
            warm = pkpool.tile([128, 128], F32, tag="pblk", name="pblk0",
                               space="PSUM")
            for _ in range(24):
                nc.tensor.matmul(warm[:], lhsT=ident[:], rhs=ident[:],
                                 start=True, stop=True)

            def load_const(name, src, shape, dt, eng=None):
                tile = cpool.tile(shape, dt, tag=name)
                (eng or nc.sync).dma_start(out=tile[:], in_=src)
                return tile

            w1_sb = load_const("w1", w1[:], [F_IN, H], F16, eng=nc.scalar)
            w2_sb = load_const("w2", w2[:], [H, H], F16, eng=nc.scalar)
            w3ab_sb = load_const("w3ab", w3[0:128, :], [128, H], F16,
                                 eng=nc.scalar)
            w3c_sb = load_const("w3c", w3[128:192, :], [H, H], F16,
                                 eng=nc.scalar)
            w4_sb = load_const("w4", w4[:], [H, NCLS], F16, eng=nc.scalar)
            b1_sb = load_const("b1", b1[:], [H, 1], F32, eng=nc.scalar)
            b2_sb = load_const("b2", b2[:], [H, 1], F32, eng=nc.scalar)
            b3_sb = load_const("b3", b3[:], [H, 1], F32, eng=nc.scalar)
            b4_sb = load_const("b4", b4[:], [NCLS, 1], F32, eng=nc.scalar)
            # big SpMM consts go via the Activation HWDGE queue so phase-A
            # xT chunk loads on the sync queue aren't stuck behind them
            idx_sb = load_const("idx", idx_t[:], [128, T * 8], I16,
                                eng=nc.scalar)
            rowv_sb = load_const("rowv", rowv_t[:], [128, T], F16,
                                 eng=nc.scalar)
            vm_sb = load_const("vm", vm_t[:], [128, 2 * T], F16,
                               eng=nc.scalar)
            iota_sb = load_const("iota", iota_t[:], [128, 256], F16,
                                 eng=nc.scalar)

            h1_f2 = cpool.tile([128, RP], F16)   # h1, then f2 (fm) on p0..63
            h_cat = cpool.tile([128, RP], F16)   # f0 fm p0..63, f1 fm p64..127
            f_rm0 = cpool.tile([128, NB * H], F16)
            f_rm1 = cpool.tile([128, NB * H], F16)
            part = cpool.tile([128, NB * H], F16)  # sub-pass-0 partial A.f

            def ship(src_rm, i, h):
                c0, c1 = HBS[h] * H, HBS[h + 1] * H
                nc.sync.dma_start(out=ag_in[i][h][:], in_=src_rm[:, c0:c1])
                nc.gpsimd.collective_compute(
                    "AllGather", mybir.AluOpType.bypass, replica_groups=rg,
                    ins=[ag_in[i][h][:]], outs=[ag_out[i][h][:]])

            # ---------- MLP1 + MLP2 + ship f0 (row-major fp16) ----------
            for o in range(0, RP, PCH):
                pc = min(PCH, RP - o)
                xt = mpool.tile([F_IN, PCH], F16, tag="xin")
                nc.sync.dma_start(out=xt[:, :pc], in_=xT[:, o:o + pc])
                pt = pspool.tile([H, PCH], F32, tag="pmlp", space="PSUM")
                nc.tensor.matmul(pt[:, :pc], lhsT=w1_sb[:], rhs=xt[:, :pc],
                                 start=True, stop=True)
                nc.vector.tensor_scalar(
                    out=h1_f2[0:H, o:o + pc], in0=pt[:, :pc],
                    scalar1=b1_sb[:], scalar2=0.0,
                    op0=mybir.AluOpType.add, op1=mybir.AluOpType.max)
                pt2 = pspool.tile([H, PCH], F32, tag="pmlp", space="PSUM")
                nc.tensor.matmul(pt2[:, :pc], lhsT=w2_sb[:],
                                 rhs=h1_f2[0:H, o:o + pc],
                                 start=True, stop=True)
                nc.scalar.activation(h_cat[0:H, o:o + pc], pt2[:, :pc],
                                     mybir.ActivationFunctionType.Relu,
                                     bias=b2_sb[:], scale=1.0)
                for b in range(o // 128, (o + pc) // 128):
                    ptr = pbpool.tile([128, 128], F16, tag="ptr", space="PSUM")
                    nc.tensor.transpose(ptr[0:128, 0:H],
                                        h_cat[0:H, b * 128:(b + 1) * 128],
                                        ident[0:H, 0:H])
                    nc.vector.tensor_copy(f_rm0[:, b * H:(b + 1) * H],
                                          ptr[0:128, 0:H])
                if (o + pc) // 128 == HB0:
                    ship(f_rm0, 0, 0)
            ship(f_rm0, 0, 1)

            # ---------- SpMM pass (two sub-passes over source halves) ----
            qrr = [0]

            def spmm(srcs, cur_rm, nxt_rm, fm_out, fm_p, ship_i=None,
                     tail_cb=None):
                qv = []
                for h in range(2):
                    flat = srcs[h][:].rearrange("p x -> (p x)")
                    qv.append(flat.rearrange("(r s) -> r s", s=128))

                def do_run(bs, sbase, runs, off, qq):
                    run = [r for r in runs if r[0] == qq]
                    if not run:
                        return None, None
                    (_, qcb0, nch, goff) = run[0]
                    gt = gpool.tile([128, gmax * 128], F16, tag="gbuf")
                    g16 = gpool.tile([128, gmax * 128], F16, tag="g16")
                    for s0 in range(0, nch, QCH):
                        ns = min(QCH, nch - s0)
                        nc.gpsimd.dma_gather(
                            out_ap=gt[:, s0 * 128:(s0 + ns) * 128]
                            .rearrange("p (t e) -> p t e", e=128),
                            in_ap=qv[qq],
                            idxs_ap=idx_sb[:, qbase[qq] + 8 * (qcb0 + s0):
                                           qbase[qq] + 8 * (qcb0 + s0 + ns)],
                            num_idxs=ns * 128,
                            num_idxs_reg=ns * 128,
                            elem_size=128,
                            single_packet=True,
                            queue_num=qrr[0],
                        )
                        qrr[0] = (qrr[0] + 1) % 4
                    nc.vector.tensor_tensor(
                        out=g16[:, :nch * 128]
                        .rearrange("p (t e) -> p t e", e=H),
                        in0=gt[:, :nch * 128]
                        .rearrange("p (t e) -> p t e", e=H),
                        in1=vm_sb[:, 2 * (sbase + goff):
                                  2 * (sbase + goff + nch)]
                        .unsqueeze(2).to_broadcast([128, 2 * nch, H]),
                        op=mybir.AluOpType.mult)
                    return g16, (sbase + goff)

                def blocks_mm(bs, off, qq, g16, rvbase, pts):
                    for a, b in enumerate(bs):
                        if L[b, qq] == 0:
                            continue
                        o0 = off[(b, qq)]
                        jb0 = o0 // 128
                        jb1 = (o0 + int(L[b, qq]) - 1) // 128
                        nj = jb1 - jb0 + 1
                        bp = (b % 2) * 128
                        sblk = slpool.tile([128, SMX * 128], F16, tag="sslab")
                        nc.vector.tensor_tensor(
                            out=sblk[:, :nj * 128]
                            .rearrange("p (t r) -> p t r", r=128),
                            in0=iota_sb[:, bp:bp + 128].unsqueeze(1)
                            .to_broadcast([128, nj, 128]),
                            in1=rowv_sb[:, rvbase + jb0:rvbase + jb0 + nj]
                            .unsqueeze(2).to_broadcast([128, nj, 128]),
                            op=mybir.AluOpType.is_equal)
                        for j in range(jb0, jb1 + 1):
                            nc.tensor.matmul(
                                pts[a][:],
                                lhsT=sblk[:, (j - jb0) * 128:
                                          (j - jb0 + 1) * 128],
                                rhs=g16[:, j * 128:(j + 1) * 128],
                                start=(j == jb0), stop=(j == jb1))

                # sub-pass 0: accumulate half-0 contributions into `part`
                for bs, sbase, tcount, runs, off in groups:
                    g16, rvbase = do_run(bs, sbase, runs, off, 0)
                    pts = [pkpool.tile([128, 128], F32, tag="pblk",
                                       name=f"pblk{ai}", space="PSUM")
                           for ai in range(len(bs))]
                    if g16 is not None:
                        blocks_mm(bs, off, 0, g16, rvbase, pts)
                    for a, b in enumerate(bs):
                        if L[b, 0] == 0:
                            nc.vector.memset(part[:, b * H:(b + 1) * H], 0)
                            continue
                        nc.scalar.copy(part[:, b * H:(b + 1) * H],
                                       pts[a][:, 0:H])
                        nc.vector.tensor_tensor(
                            out=part[:, b * H:(b + 1) * H],
                            in0=part[:, b * H:(b + 1) * H],
                            in1=pts[a][:, H:128],
                            op=mybir.AluOpType.add)

                # sub-pass 1: half-1 contributions, final combine + fm copy
                for gi, (bs, sbase, tcount, runs, off) in enumerate(groups):
                    g16, rvbase = do_run(bs, sbase, runs, off, 1)
                    pts = [pkpool.tile([128, 128], F32, tag="pblk",
                                       name=f"pblk{ai}", space="PSUM")
                           for ai in range(len(bs))]
                    if g16 is not None:
                        blocks_mm(bs, off, 1, g16, rvbase, pts)
                    for a, b in enumerate(bs):
                        ts = subpool.tile([128, H], F16, tag="tsub")
                        nc.vector.tensor_tensor(
                            out=ts[:],
                            in0=cur_rm[:, b * H:(b + 1) * H],
                            in1=part[:, b * H:(b + 1) * H],
                            op=mybir.AluOpType.subtract)
                        if L[b, 1] > 0:
                            t2 = subpool.tile([128, H], F16, tag="tsu2")
                            nc.vector.tensor_tensor(
                                out=t2[:], in0=ts[:], in1=pts[a][:, 0:H],
                                op=mybir.AluOpType.subtract)
                            nc.vector.tensor_tensor(
                                out=nxt_rm[:, b * H:(b + 1) * H],
                                in0=t2[:], in1=pts[a][:, H:128],
                                op=mybir.AluOpType.subtract)
                        else:
                            nc.vector.tensor_copy(
                                nxt_rm[:, b * H:(b + 1) * H], ts[:])
                        ptr = pbpool.tile([128, 128], F16, tag="ptr",
                                          space="PSUM")
                        nc.tensor.transpose(
                            ptr[0:H, 0:128],
                            nxt_rm[:, b * H:(b + 1) * H],
                            ident[:])
                        nc.scalar.copy(
                            fm_out[fm_p:fm_p + H, b * 128:(b + 1) * 128],
                            ptr[0:H, 0:128])
                    if ship_i is not None and bs[-1] + 1 == HB0:
                        ship(nxt_rm, ship_i, 0)
                    if tail_cb is not None:
                        tail_cb(bs[-1] + 1)
                if ship_i is not None:
                    ship(nxt_rm, ship_i, 1)

            # ---------- MLP3 + MLP4 (interleaved into pass 2's tail) ----
            PC3 = 256
            m34_done = [0]

            def mlp34_upto(blocks_done):
                hi = blocks_done * 128 // PC3
                for c3 in range(m34_done[0], hi):
                    o = c3 * PC3
                    pt = pspool.tile([H, PC3], F32, tag="pmlp", space="PSUM")
                    nc.tensor.matmul(pt[:], lhsT=w3ab_sb[:],
                                     rhs=h_cat[:, o:o + PC3],
                                     start=True, stop=False)
                    nc.tensor.matmul(pt[:], lhsT=w3c_sb[:],
                                     rhs=h1_f2[0:H, o:o + PC3],
                                     start=False, stop=True)
                    h3 = mpool.tile([H, PC3], F16, tag="h3")
                    nc.scalar.activation(h3[:], pt[:],
                                         mybir.ActivationFunctionType.Relu,
                                         bias=b3_sb[:], scale=1.0)
                    po = pspool.tile([H, PC3], F32, tag="pmlp", space="PSUM")
                    nc.tensor.matmul(po[0:NCLS, :], lhsT=w4_sb[:], rhs=h3[:],
                                     start=True, stop=True)
                    ot = mpool.tile([NCLS, PC3], F32, tag="ot")
                    nc.scalar.activation(ot[:], po[0:NCLS, :],
                                         mybir.ActivationFunctionType.Identity,
                                         bias=b4_sb[:], scale=1.0)
                    nc.sync.dma_start(out=out_t[:, o:o + PC3], in_=ot[:])
                m34_done[0] = hi

            spmm(ag_out[0], f_rm0, f_rm1, h_cat, H, ship_i=1)
            spmm(ag_out[1], f_rm1, f_rm0, h1_f2, 0, tail_cb=mlp34_upto)
            mlp34_upto(NB)

    nc.compile()
    return nc


def _plan(in_feat, adj_rows, adj_cols, adj_vals, W1, b1, W2, b2, W3, b3, W4, b4):
    in_feat = np.asarray(in_feat, dtype=np.float32)
    adj_rows = np.asarray(adj_rows).astype(np.int64)
    adj_cols = np.asarray(adj_cols).astype(np.int64)
    adj_vals = np.asarray(adj_vals, dtype=np.float32)

    thetas = _theta2()
    W3 = np.asarray(W3, dtype=np.float64)
    W3p = np.zeros((3 * H, H), dtype=np.float64)
    for k in range(D + 1):
        for t in range(D + 1):
            W3p[k * H:(k + 1) * H] += thetas[t][k] * W3[t * H:(t + 1) * H]

    idx_wrapped, rowv, vm, groups, L, T, Tq, newpos = _prep_edges(
        adj_rows, adj_cols, adj_vals)

    nc = _build(groups, L, T, Tq)

    iota = np.tile(np.arange(256, dtype=np.float16), (128, 1))
    in_maps = []
    for c in range(W):
        shard = np.zeros((F_IN, RP), dtype=np.float16)
        shard[:, newpos[c]] = in_feat[c * R:(c + 1) * R].T.astype(np.float16)
        in_maps.append({
            "xT": shard,
            "w1": np.asarray(W1).astype(np.float16),
            "w2": np.asarray(W2).astype(np.float16),
            "w3": W3p.astype(np.float16),
            "w4": np.asarray(W4).astype(np.float16),
            "b1": np.asarray(b1, dtype=np.float32).reshape(H, 1),
            "b2": np.asarray(b2, dtype=np.float32).reshape(H, 1),
            "b3": np.asarray(b3, dtype=np.float32).reshape(H, 1),
            "b4": np.asarray(b4, dtype=np.float32).reshape(NCLS, 1),
            "idx": idx_wrapped[c],
            "rowv": rowv[c],
            "vm": vm[c],
            "iota": iota,
        })
    return nc, in_maps, newpos


def kernel(in_feat, adj_rows, adj_cols, adj_vals, W1, b1, W2, b2, W3, b3, W4, b4):
    nc, in_maps, newpos = _plan(in_feat, adj_rows, adj_cols, adj_vals,
                                W1, b1, W2, b2, W3, b3, W4, b4)
    res = bass_utils.run_bass_kernel_spmd(nc, in_maps, list(range(W)))
    out = np.concatenate(
        [res.results[c]["out"][:, newpos[c]].T for c in range(W)], axis=0)
    return np.ascontiguousarray(out, dtype=np.float32)


# revision 64
# speedup vs baseline: 1.1774x; 1.0141x over previous
"""BWGNN (Bernstein-polynomial graph conv, D=2) on 8 Trainium2 NeuronCores.

Key algebra: inside each polyconv the feat sequence f0, f1=f0-A f0, f2=f1-A f1
is theta-independent, so the device needs only TWO SpMMs (not six), and the
Bernstein mixing folds into W3 on the host:
    out = relu([f0|f1|f2] @ W3' + b3) @ W4 + b4,
    W3'[k*H+i, j] = sum_t theta[t][k] * W3[t*H+i, j].

Distribution (8 cores, SPMD single program):
- nodes row-sharded: core c owns rows [c*12500, (c+1)*12500), padded to 12544.
- node-feature tables are ROW-major fp16 [128, NB*H]; flat row (p, b) holds
  node rl = b*128+p.  The table ships in TWO AllGather halves split by
  source-block range ({0..47}, {48..97}) so each half can fire as soon as the
  producing compute finishes and the next consumer only ever waits on the
  half it reads -- the gather stream never drains at phase boundaries.
- SpMM gathers PAIRS of table rows (256B = two nodes' 64 fp16 feats) per
  edge; a [128, 2] val-mask (val at the edge's parity half, 0 at the other)
  folds value scaling AND pair selection into one DVE multiply.
- matmuls are ROW-major: out[dest, feat] = S^T @ G with lhsT=S (one-hot
  dest-row selector, 128 wide, built per (block, run) with a parity-offset
  iota so chunks spanning two blocks just get two full-partition matmuls).
- each pass runs as two sub-passes (source half 0 for all groups, then half
  1) accumulating half-0 results into an SBUF partial, so sub-pass 1's
  gathers are the only ops waiting on the second AllGather half.
- edges are packed per (4-block group, half) run: per-(block,half) slot
  budgets are equalized across cores (SPMD); chunks are not rounded per
  block (~10% padding instead of ~25%).
"""
import math
import numpy as np

import concourse.bass as bass
import concourse.bacc as bacc
import concourse.mybir as mybir
from concourse.tile import TileContext
from concourse.masks import make_identity
from concourse import bass_utils

N = 100000
F_IN = 128
H = 64
NCLS = 2
D = 2
W = 8                   # cores
R = 12500               # real rows per core
RP = 12544              # padded rows per core (98 * 128)
NB = 98                 # dest blocks per core
NQ = 2                  # source halves (block ranges)
HB0 = 48                # blocks in half 0 (even => pairs don't straddle)
HBS = [0, HB0, NB]      # half boundaries
QS = [W * 128 * HB0 // 2, W * 128 * (NB - HB0) // 2]   # pairs per half
GSC = 4                 # dest blocks per group
PCH = 256               # dense-layer column chunk (2 blocks)
F16 = mybir.dt.float16
F32 = mybir.dt.float32
I16 = mybir.dt.int16


def _theta2():
    P = np.polynomial.polynomial
    thetas = []
    for i in range(D + 1):
        beta = math.factorial(i) * math.factorial(D - i) / math.factorial(D + 1)
        c = P.polymul(P.polypow([0.0, 0.5], i), P.polypow([1.0, -0.5], D - i)) / beta
        c = np.pad(c, (0, D + 1 - len(c)))
        thetas.append(c.astype(np.float64))
    return thetas


def _prep_schedule(counts):
    """counts: [W, NB, NQ] per-core edge counts -> static schedule.

    groups entries (bs, sbase, tcount, runs, off):
      runs = [(q, qcb0, nch, goff)], off[(b, q)] = slot offset within run.
    """
    L = counts.max(axis=0).astype(np.int64)
    for b in range(NB):
        if L[b].sum() == 0:
            L[b, 0] = 1
    groups = []
    sbase = 0
    qcb = np.zeros(NQ, dtype=np.int64)
    for g0 in range(0, NB, GSC):
        bs = list(range(g0, min(g0 + GSC, NB)))
        runs = []
        off = {}
        goff = 0
        for q in range(NQ):
            tot = 0
            for b in bs:
                off[(b, q)] = tot
                tot += int(L[b, q])
            nch = (tot + 127) // 128
            if nch == 0:
                continue
            runs.append((q, int(qcb[q]), nch, goff))
            qcb[q] += nch
            goff += nch
        groups.append((bs, sbase, goff, runs, off))
        sbase += goff
    return L, groups, sbase, [int(qcb[q]) for q in range(NQ)]


def _balance_nodes(adj_rows, adj_cols):
    """Within-core node permutation flattening per-(block, half) in-degree
    sums across cores.  Nodes stay within their source half, so edge half
    assignments are invariant and no iteration is needed.  Returns
    newpos[W, R] (new padded-position of local node rl)."""
    core = adj_rows // R
    rloc = adj_rows - core * R
    qe = ((adj_cols % R) // 128 >= HB0).astype(np.int64)
    d = np.zeros((W, R, NQ), dtype=np.int64)
    np.add.at(d, (core, rloc, qe), 1)
    newpos = np.zeros((W, R), dtype=np.int64)
    for c in range(W):
        for b0, b1 in [(0, HB0), (HB0, NB)]:
            lo, hi = b0 * 128, min(b1 * 128, R)
            dd = d[c, lo:hi]
            order = np.lexsort((dd[:, 0] - dd[:, 1],
                                -(dd[:, 0] + dd[:, 1])))
            caps = [min((b + 1) * 128, R) - b * 128 for b in range(b0, b1)]
            seq = []
            fill = [0] * len(caps)
            rev = False
            while len(seq) < hi - lo:
                rng = range(len(caps) - 1, -1, -1) if rev else range(len(caps))
                for bi in rng:
                    if fill[bi] < caps[bi]:
                        seq.append((bi, fill[bi]))
                        fill[bi] += 1
                rev = not rev
            bi_arr = np.array([s[0] for s in seq], dtype=np.int64)
            sl_arr = np.array([s[1] for s in seq], dtype=np.int64)
            newpos[c, lo + order] = (b0 + bi_arr) * 128 + sl_arr
    return newpos


def _prep_edges(adj_rows, adj_cols, adj_vals):
    """Per-core gather indices / dest-row vectors / val-masks in slot order.

    Source node (c, rl) at padded position np=newpos[c,rl], p=np%128,
    b=np//128 lives at flat row (c*128+p)*HBn + (b-b0) of its half's
    table; pair = row//2, parity = b%2 (half sizes even)."""
    newpos = _balance_nodes(adj_rows, adj_cols)
    core = adj_rows // R
    rloc = adj_rows - core * R
    dpos = newpos[core, rloc]
    blk = dpos // 128
    rowin = dpos % 128
    csrc = adj_cols // R
    rsrc = adj_cols - csrc * R
    spos = newpos[csrc, rsrc]
    bsrc = spos // 128
    cp = csrc * 128 + spos % 128
    q = (bsrc >= HB0).astype(np.int64)
    hbn = np.where(q == 0, HB0, NB - HB0)
    row = cp * hbn + bsrc - np.where(q == 0, 0, HB0)
    pair = row // 2
    parity = bsrc % 2
    qoff = pair

    counts = np.zeros((W, NB, NQ), dtype=np.int64)
    np.add.at(counts, (core, blk, q), 1)
    L, groups, T, Tq = _prep_schedule(counts)

    off_arr = np.zeros((NB, NQ), dtype=np.int64)
    runbase = np.full((NB, NQ), -1, dtype=np.int64)
    qchunkbase = np.zeros((NB, NQ), dtype=np.int64)
    for bs, sbase, tcount, runs, off in groups:
        for (qq, qcb0, nch, goff) in runs:
            for b in bs:
                off_arr[b, qq] = off[(b, qq)]
                runbase[b, qq] = sbase + goff
                qchunkbase[b, qq] = qcb0

    order = np.lexsort((qoff, blk, q, core))
    sc, sb_, sq = core[order], blk[order], q[order]
    s_qoff, s_rowin = qoff[order], rowin[order]
    s_par, s_val = parity[order], adj_vals[order]

    m = len(sc)
    key = (sc * NB + sb_) * NQ + sq
    brk = np.nonzero(np.diff(key))[0] + 1
    starts = np.concatenate([[0], brk])
    lens = np.diff(np.concatenate([starts, [m]]))
    pos = np.arange(m) - np.repeat(starts, lens)

    runslot = off_arr[sb_, sq] + pos
    jq = runslot // 128
    p = runslot % 128
    tglob = runbase[sb_, sq] + jq
    qch = qchunkbase[sb_, sq] + jq
    assert runbase[sb_, sq].min() >= 0
    assert int(qoff.max()) < max(QS) <= 32768

    rowv = np.zeros((W, 128, T), dtype=np.float16)
    vm = np.zeros((W, 128, 2 * T), dtype=np.float16)
    idx16 = [[np.zeros(Tq[qq] * 128, dtype=np.int16) for qq in range(NQ)]
             for _ in range(W)]
    s_rowd = s_rowin + 128 * (sb_ % 2)      # parity-disambiguated dest row
    for c in range(W):
        sel = sc == c
        rowv[c, p[sel], tglob[sel]] = s_rowd[sel].astype(np.float16)
        vm[c, p[sel], 2 * tglob[sel] + s_par[sel]] = s_val[sel].astype(np.float16)
        for qq in range(NQ):
            s2 = sel & (sq == qq)
            idx16[c][qq][qch[s2] * 128 + p[s2]] = s_qoff[s2].astype(np.int16)

    idx_wrapped = []
    for c in range(W):
        parts = []
        for qq in range(NQ):
            a = idx16[c][qq]
            parts.append(a.reshape(len(a) // 16, 16).T)
        cat = np.concatenate(parts, axis=1)
        idx_wrapped.append(np.tile(cat, (8, 1)).copy())
    return idx_wrapped, rowv, vm, groups, L, T, Tq, newpos


def _build(groups, L, T, Tq):
    qbase = [8 * sum(Tq[:qq]) for qq in range(NQ)]
    gmax = max(nch for _, _, _, runs, _ in groups for (_, _, nch, _) in runs)
    QCH = 4                              # chunks per gather call (queue spread)
    SMX = 1 + max((int(L[b, q]) + 127) // 128
                  for b in range(NB) for q in range(NQ))
    CW = [HB0 * H, (NB - HB0) * H]       # rm column widths of the two halves

    nc = bacc.Bacc("TRN2", num_swdge_queues=4)
    rg = [list(range(W))]

    xT = nc.dram_tensor("xT", [F_IN, RP], F16, kind="ExternalInput")
    w1 = nc.dram_tensor("w1", [F_IN, H], F16, kind="ExternalInput")
    w2 = nc.dram_tensor("w2", [H, H], F16, kind="ExternalInput")
    w3 = nc.dram_tensor("w3", [3 * H, H], F16, kind="ExternalInput")
    w4 = nc.dram_tensor("w4", [H, NCLS], F16, kind="ExternalInput")
    b1 = nc.dram_tensor("b1", [H, 1], F32, kind="ExternalInput")
    b2 = nc.dram_tensor("b2", [H, 1], F32, kind="ExternalInput")
    b3 = nc.dram_tensor("b3", [H, 1], F32, kind="ExternalInput")
    b4 = nc.dram_tensor("b4", [NCLS, 1], F32, kind="ExternalInput")
    idx_t = nc.dram_tensor("idx", [128, T * 8], I16, kind="ExternalInput")
    rowv_t = nc.dram_tensor("rowv", [128, T], F16, kind="ExternalInput")
    vm_t = nc.dram_tensor("vm", [128, 2 * T], F16, kind="ExternalInput")
    iota_t = nc.dram_tensor("iota", [128, 256], F16, kind="ExternalInput")
    out_t = nc.dram_tensor("out", [NCLS, RP], F32, kind="ExternalOutput")

    ag_in = [[nc.dram_tensor(f"agin{i}{h}", [128, CW[h]], F16, kind="Internal")
              for h in range(2)] for i in range(2)]
    ag_out = [[nc.dram_tensor(f"agout{i}{h}", [W * 128, CW[h]], F16,
                              kind="Internal", addr_space="Shared")
               for h in range(2)] for i in range(2)]

    with TileContext(nc) as tc:
        with tc.tile_pool(name="c0", bufs=1) as cpool, \
             tc.tile_pool(name="mm", bufs=3) as mpool, \
             tc.tile_pool(name="gg", bufs=5) as gpool, \
             tc.tile_pool(name="sl", bufs=6) as slpool, \
             tc.tile_pool(name="sb", bufs=8) as subpool, \
             tc.tile_pool(name="ps", bufs=2, space="PSUM") as pspool, \
             tc.tile_pool(name="pb", bufs=2, space="PSUM") as pbpool, \
             tc.tile_pool(name="pk", bufs=4, space="PSUM") as pkpool:

            ident = cpool.tile([128, 128], F16)
            make_identity(nc, ident[:])

            # PE pre-warm: ~5us of junk matmuls lifts the HAM clock gate to
            # 2.4 GHz before the MLP stream begins (see tensor-engine docs).
            warm = pkpool.tile([128, 128], F32, tag="pblk", name="pblk0",
                               space="PSUM")
            for _ in range(24):
                nc.tensor.matmul(warm[:], lhsT=ident[:], rhs=ident[:],
                                 start=True, stop=True)

            def load_const(name, src, shape, dt, eng=None):
                tile = cpool.tile(shape, dt, tag=name)
                (eng or nc.sync).dma_start(out=tile[:], in_=src)
                return tile

            w1_sb = load_const("w1", w1[:], [F_IN, H], F16, eng=nc.scalar)
            w2_sb = load_const("w2", w2[:], [H, H], F16, eng=nc.scalar)
            w3ab_sb = load_const("w3ab", w3[0:128, :], [128, H], F16,
                                 eng=nc.scalar)
            w3c_sb = load_const("w3c", w3[128:192, :], [H, H], F16,
                                 eng=nc.scalar)
            w4_sb = load_const("w4", w4[:], [H, NCLS], F16, eng=nc.scalar)
            b1_sb = load_const("b1", b1[:], [H, 1], F32, eng=nc.scalar)
            b2_sb = load_const("b2", b2[:], [H, 1], F32, eng=nc.scalar)
            b3_sb = load_const("b3", b3[:], [H, 1], F32, eng=nc.scalar)
            b4_sb = load_const("b4", b4[:], [NCLS, 1], F32, eng=nc.scalar)
            # big SpMM consts go via the Activation HWDGE queue so phase-A
            # xT chunk loads on the sync queue aren't stuck behind them
            idx_sb = load_const("idx", idx_t[:], [128, T * 8], I16,
                                eng=nc.scalar)
            rowv_sb = load_const("rowv", rowv_t[:], [128, T], F16,
                                 eng=nc.scalar)
            vm_sb = load_const("vm", vm_t[:], [128, 2 * T], F16,
                               eng=nc.scalar)
            iota_sb = load_const("iota", iota_t[:], [128, 256], F16,
                                 eng=nc.scalar)

            h1_f2 = cpool.tile([128, RP], F16)   # h1, then f2 (fm) on p0..63
            h_cat = cpool.tile([128, RP], F16)   # f0 fm p0..63, f1 fm p64..127
            f_rm0 = cpool.tile([128, NB * H], F16)
            f_rm1 = cpool.tile([128, NB * H], F16)
            part = cpool.tile([128, NB * H], F16)  # sub-pass-0 partial A.f

            def ship(src_rm, i, h):
                c0, c1 = HBS[h] * H, HBS[h + 1] * H
                nc.sync.dma_start(out=ag_in[i][h][:], in_=src_rm[:, c0:c1])
                nc.gpsimd.collective_compute(
                    "AllGather", mybir.AluOpType.bypass, replica_groups=rg,
                    ins=[ag_in[i][h][:]], outs=[ag_out[i][h][:]])

            # ---------- MLP1 + MLP2 + ship f0 (row-major fp16) ----------
            for o in range(0, RP, PCH):
                pc = min(PCH, RP - o)
                xt = mpool.tile([F_IN, PCH], F16, tag="xin")
                nc.sync.dma_start(out=xt[:, :pc], in_=xT[:, o:o + pc])
                pt = pspool.tile([H, PCH], F32, tag="pmlp", space="PSUM")
                nc.tensor.matmul(pt[:, :pc], lhsT=w1_sb[:], rhs=xt[:, :pc],
                                 start=True, stop=True)
                nc.vector.tensor_scalar(
                    out=h1_f2[0:H, o:o + pc], in0=pt[:, :pc],
                    scalar1=b1_sb[:], scalar2=0.0,
                    op0=mybir.AluOpType.add, op1=mybir.AluOpType.max)
                pt2 = pspool.tile([H, PCH], F32, tag="pmlp", space="PSUM")
                nc.tensor.matmul(pt2[:, :pc], lhsT=w2_sb[:],
                                 rhs=h1_f2[0:H, o:o + pc],
                                 start=True, stop=True)
                nc.scalar.activation(h_cat[0:H, o:o + pc], pt2[:, :pc],
                                     mybir.ActivationFunctionType.Relu,
                                     bias=b2_sb[:], scale=1.0)
                for b in range(o // 128, (o + pc) // 128):
                    ptr = pbpool.tile([128, 128], F16, tag="ptr", space="PSUM")
                    nc.tensor.transpose(ptr[0:128, 0:H],
                                        h_cat[0:H, b * 128:(b + 1) * 128],
                                        ident[0:H, 0:H])
                    nc.vector.tensor_copy(f_rm0[:, b * H:(b + 1) * H],
                                          ptr[0:128, 0:H])
                if (o + pc) // 128 == HB0:
                    ship(f_rm0, 0, 0)
            ship(f_rm0, 0, 1)

            # ---------- SpMM pass (two sub-passes over source halves) ----
            qrr = [0]

            def spmm(srcs, cur_rm, nxt_rm, fm_out, fm_p, ship_i=None,
                     tail_cb=None):
                qv = []
                for h in range(2):
                    flat = srcs[h][:].rearrange("p x -> (p x)")
                    qv.append(flat.rearrange("(r s) -> r s", s=128))

                def do_run(bs, sbase, runs, off, qq):
                    run = [r for r in runs if r[0] == qq]
                    if not run:
                        return None, None
                    (_, qcb0, nch, goff) = run[0]
                    gt = gpool.tile([128, gmax * 128], F16, tag="gbuf")
                    g16 = gpool.tile([128, gmax * 128], F16, tag="g16")
                    for s0 in range(0, nch, QCH):
                        ns = min(QCH, nch - s0)
                        nc.gpsimd.dma_gather(
                            out_ap=gt[:, s0 * 128:(s0 + ns) * 128]
                            .rearrange("p (t e) -> p t e", e=128),
                            in_ap=qv[qq],
                            idxs_ap=idx_sb[:, qbase[qq] + 8 * (qcb0 + s0):
                                           qbase[qq] + 8 * (qcb0 + s0 + ns)],
                            num_idxs=ns * 128,
                            num_idxs_reg=ns * 128,
                            elem_size=128,
                            single_packet=True,
                            queue_num=qrr[0],
                        )
                        qrr[0] = (qrr[0] + 1) % 4
                    nc.vector.tensor_tensor(
                        out=g16[:, :nch * 128]
                        .rearrange("p (t e) -> p t e", e=H),
                        in0=gt[:, :nch * 128]
                        .rearrange("p (t e) -> p t e", e=H),
                        in1=vm_sb[:, 2 * (sbase + goff):
                                  2 * (sbase + goff + nch)]
                        .unsqueeze(2).to_broadcast([128, 2 * nch, H]),
                        op=mybir.AluOpType.mult)
                    return g16, (sbase + goff)

                def blocks_mm(bs, off, qq, g16, rvbase, pts):
                    for a, b in enumerate(bs):
                        if L[b, qq] == 0:
                            continue
                        o0 = off[(b, qq)]
                        jb0 = o0 // 128
                        jb1 = (o0 + int(L[b, qq]) - 1) // 128
                        nj = jb1 - jb0 + 1
                        bp = (b % 2) * 128
                        sblk = slpool.tile([128, SMX * 128], F16, tag="sslab")
                        nc.vector.tensor_tensor(
                            out=sblk[:, :nj * 128]
                            .rearrange("p (t r) -> p t r", r=128),
                            in0=iota_sb[:, bp:bp + 128].unsqueeze(1)
                            .to_broadcast([128, nj, 128]),
                            in1=rowv_sb[:, rvbase + jb0:rvbase + jb0 + nj]
                            .unsqueeze(2).to_broadcast([128, nj, 128]),
                            op=mybir.AluOpType.is_equal)
                        for j in range(jb0, jb1 + 1):
                            nc.tensor.matmul(
                                pts[a][:],
                                lhsT=sblk[:, (j - jb0) * 128:
                                          (j - jb0 + 1) * 128],
                                rhs=g16[:, j * 128:(j + 1) * 128],
                                start=(j == jb0), stop=(j == jb1))

                # sub-pass 0: accumulate half-0 contributions into `part`
                for bs, sbase, tcount, runs, off in groups:
                    g16, rvbase = do_run(bs, sbase, runs, off, 0)
                    pts = [pkpool.tile([128, 128], F32, tag="pblk",
                                       name=f"pblk{ai}", space="PSUM")
                           for ai in range(len(bs))]
                    if g16 is not None:
                        blocks_mm(bs, off, 0, g16, rvbase, pts)
                    for a, b in enumerate(bs):
                        if L[b, 0] == 0:
                            nc.vector.memset(part[:, b * H:(b + 1) * H], 0)
                            continue
                        nc.scalar.copy(part[:, b * H:(b + 1) * H],
                                       pts[a][:, 0:H])
                        nc.vector.tensor_tensor(
                            out=part[:, b * H:(b + 1) * H],
                            in0=part[:, b * H:(b + 1) * H],
                            in1=pts[a][:, H:128],
                            op=mybir.AluOpType.add)

                # sub-pass 1: half-1 contributions, final combine + fm copy
                for gi, (bs, sbase, tcount, runs, off) in enumerate(groups):
                    g16, rvbase = do_run(bs, sbase, runs, off, 1)
                    pts = [pkpool.tile([128, 128], F32, tag="pblk",
                                       name=f"pblk{ai}", space="PSUM")
                           for ai in range(len(bs))]
                    if g16 is not None:
                        blocks_mm(bs, off, 1, g16, rvbase, pts)
                    for a, b in enumerate(bs):
                        ts = subpool.tile([128, H], F16, tag="tsub")
                        nc.vector.tensor_tensor(
                            out=ts[:],
                            in0=cur_rm[:, b * H:(b + 1) * H],
                            in1=part[:, b * H:(b + 1) * H],
                            op=mybir.AluOpType.subtract)
                        if L[b, 1] > 0:
                            t2 = subpool.tile([128, H], F16, tag="tsu2")
                            nc.vector.tensor_tensor(
                                out=t2[:], in0=ts[:], in1=pts[a][:, 0:H],
                                op=mybir.AluOpType.subtract)
                            nc.vector.tensor_tensor(
                                out=nxt_rm[:, b * H:(b + 1) * H],
                                in0=t2[:], in1=pts[a][:, H:128],
                                op=mybir.AluOpType.subtract)
                        else:
                            nc.vector.tensor_copy(
                                nxt_rm[:, b * H:(b + 1) * H], ts[:])
                        ptr = pbpool.tile([128, 128], F16, tag="ptr",
                                          space="PSUM")
                        nc.tensor.transpose(
                            ptr[0:H, 0:128],
                            nxt_rm[:, b * H:(b + 1) * H],
                            ident[:])
                        nc.scalar.copy(
                            fm_out[fm_p:fm_p + H, b * 128:(b + 1) * 128],
                            ptr[0:H, 0:128])
                    if ship_i is not None and bs[-1] + 1 == HB0:
                        ship(nxt_rm, ship_i, 0)
                    if tail_cb is not None:
                        tail_cb(bs[-1] + 1)
                if ship_i is not None:
                    ship(nxt_rm, ship_i, 1)

            # ---------- MLP3 + MLP4 (interleaved into pass 2's tail) ----
            PC3 = 256
            m34_done = [0]

            def mlp34_upto(blocks_done):
                hi = blocks_done * 128 // PC3
                for c3 in range(m34_done[0], hi):
                    o = c3 * PC3
                    pt = pspool.tile([H, PC3], F32, tag="pmlp", space="PSUM")
                    nc.tensor.matmul(pt[:], lhsT=w3ab_sb[:],
                                     rhs=h_cat[:, o:o + PC3],
                                     start=True, stop=False)
                    nc.tensor.matmul(pt[:], lhsT=w3c_sb[:],
                                     rhs=h1_f2[0:H, o:o + PC3],
                                     start=False, stop=True)
                    h3 = mpool.tile([H, PC3], F16, tag="h3")
                    nc.scalar.activation(h3[:], pt[:],
                                         mybir.ActivationFunctionType.Relu,
                                         bias=b3_sb[:], scale=1.0)
                    po = pspool.tile([H, PC3], F32, tag="pmlp", space="PSUM")
                    nc.tensor.matmul(po[0:NCLS, :], lhsT=w4_sb[:], rhs=h3[:],
                                     start=True, stop=True)
                    ot = mpool.tile([NCLS, PC3], F32, tag="ot")
                    nc.scalar.activation(ot[:], po[0:NCLS, :],
                                         mybir.ActivationFunctionType.Identity,
                                         bias=b4_sb[:], scale=1.0)
                    nc.sync.dma_start(out=out_t[:, o:o + PC3], in_=ot[:])
                m34_done[0] = hi

            spmm(ag_out[0], f_rm0, f_rm1, h_cat, H, ship_i=1)
            spmm(ag_out[1], f_rm1, f_rm0, h1_f2, 0, tail_cb=mlp34_upto)
            mlp34_upto(NB)

    nc.compile()
    return nc


def _plan(in_feat, adj_rows, adj_cols, adj_vals, W1, b1, W2, b2, W3, b3, W4, b4):
    in_feat = np.asarray(in_feat, dtype=np.float32)
    adj_rows = np.asarray(adj_rows).astype(np.int64)
    adj_cols = np.asarray(adj_cols).astype(np.int64)
    adj_vals = np.asarray(adj_vals, dtype=np.float32)

    thetas = _theta2()
    W3 = np.asarray(W3, dtype=np.float64)
    W3p = np.zeros((3 * H, H), dtype=np.float64)
    for k in range(D + 1):
        for t in range(D + 1):
            W3p[k * H:(k + 1) * H] += thetas[t][k] * W3[t * H:(t + 1) * H]

    idx_wrapped, rowv, vm, groups, L, T, Tq, newpos = _prep_edges(
        adj_rows, adj_cols, adj_vals)

    nc = _build(groups, L, T, Tq)

    iota = np.tile(np.arange(256, dtype=np.float16), (128, 1))
    in_maps = []
    for c in range(W):
        shard = np.zeros((F_IN, RP), dtype=np.float16)
        shard[:, newpos[c]] = in_feat[c * R:(c + 1) * R].T.astype(np.float16)
        in_maps.append({
            "xT": shard,
            "w1": np.asarray(W1).astype(np.float16),
            "w2": np.asarray(W2).astype(np.float16),
            "w3": W3p.astype(np.float16),
            "w4": np.asarray(W4).astype(np.float16),
            "b1": np.asarray(b1, dtype=np.float32).reshape(H, 1),
            "b2": np.asarray(b2, dtype=np.float32).reshape(H, 1),
            "b3": np.asarray(b3, dtype=np.float32).reshape(H, 1),
            "b4": np.asarray(b4, dtype=np.float32).reshape(NCLS, 1),
            "idx": idx_wrapped[c],
            "rowv": rowv[c],
            "vm": vm[c],
            "iota": iota,
        })
    return nc, in_maps, newpos


def kernel(in_feat, adj_rows, adj_cols, adj_vals, W1, b1, W2, b2, W3, b3, W4, b4):
    nc, in_maps, newpos = _plan(in_feat, adj_rows, adj_cols, adj_vals,
                                W1, b1, W2, b2, W3, b3, W4, b4)
    res = bass_utils.run_bass_kernel_spmd(nc, in_maps, list(range(W)))
    out = np.concatenate(
        [res.results[c]["out"][:, newpos[c]].T for c in range(W)], axis=0)
    return np.ascontiguousarray(out, dtype=np.float32)
